# revision 32
# baseline (speedup 1.0000x reference)
"""GIN message-passing (2 GINConv layers + 2 linear) on 8 TRN2 NeuronCores.

Strategy (self-contained; shapes hardcoded for the 100k-node / 1.6M-edge
problem):
  - Shard dst nodes across 8 cores (12500 each). Each core owns the edges
    whose dst is in its shard; dst tiles of 128 nodes.
  - Per (batch, group), source rows are gathered with gpsimd.dma_gather
    (int16 indices; node table split into <=32768-row groups; <=1024 idx per
    call round-robined over 4 SWDGE queues). Real slots are packed densely
    (no per-tile alignment); trailing pad slots use idx=-1 and are skipped
    via num_idxs_reg, so pad slots cost no DMA descriptors.
  - Scatter-add into per-tile PSUM aggregates via one-hot matmuls. A chunk
    of 128 slots may span several dst tiles; each (chunk, tile) pair gets
    its own dstloc column (others masked to -1), so boundary chunks are
    matmul'd once per spanned tile.
  - Layer 1 gathers x in fp16 (256B rows), computes z = relu((x + A@x)@W1
    + b1) @ W2 per shard, writes z rows fp16 (padded to 128 ch) to HBM; a
    sliced AllGather (overlapped with phase 1) shares z.
  - Layer 2 gathers z rows (fp16 256B), h2 = relu(z_dst + A@z + b2),
    h3 = relu(h2@W3+b3), out = h3@W4+b4. Only the 64 real z channels feed
    the aggregation matmuls (lhsT [128, 64]).
"""

import numpy as np

P = 128
WIN = 1024  # max idx per dma_gather call


class Cfg:
    def __init__(self, n_nodes, n_edges, in_ch, hid, n_cores, t_b, n_slices=4):
        self.N = n_nodes
        self.E = n_edges
        self.CH = in_ch
        self.H = hid
        self.NCORE = n_cores
        self.SHARD = n_nodes // n_cores
        self.NT = -(-self.SHARD // P)
        self.NQ = 4  # x16 table groups (quarters)
        self.QS = n_nodes // 4
        assert self.QS <= 32768
        self.NS = n_slices  # z table groups (AG slices)
        assert self.SHARD % n_slices == 0
        self.SROW = self.SHARD // n_slices  # shard rows per slice
        assert self.SROW * n_cores <= 32768
        self.T_B = t_b
        self.NB = -(-self.NT // t_b)


FULL = Cfg(100000, 1600000, 128, 64, 8, 5, n_slices=4)


def _r128(a):
    return ((a + 127) // 128) * 128


class LayerPlan:
    """Gather schedule for one layer.

    Per (batch, group): tiles' real slots (max over cores) packed densely,
    rounded up to 128 with trailing skip-pads; 1024-idx gather windows; and
    per-(chunk, tile) one-hot instances.
    """

    def __init__(self, cfg, n_groups, per_core_edges):
        c = cfg
        self.cfg = c
        self.NG = n_groups
        counts = np.zeros((c.NCORE, c.NT, n_groups), dtype=np.int64)
        self.groups = []
        for ci in range(c.NCORE):
            t, grp, lidx, dl = per_core_edges[ci]
            key = t * n_groups + grp
            order = np.argsort(key, kind="stable")
            key = key[order]
            lidx = lidx[order].astype(np.int16)
            dl = dl[order].astype(np.int16)
            cnt = np.bincount(key, minlength=c.NT * n_groups).reshape(
                c.NT, n_groups
            )
            counts[ci] = cnt
            bounds = np.concatenate([[0], np.cumsum(cnt.reshape(-1))])
            g = {}
            for tt in range(c.NT):
                for qq in range(n_groups):
                    k = tt * n_groups + qq
                    lo, hi = bounds[k], bounds[k + 1]
                    if hi > lo:
                        g[(tt, qq)] = (lidx[lo:hi], dl[lo:hi])
            self.groups.append(g)

        self.real = counts.max(axis=0)  # [NT, NG] shared geometry
        self.batches = []
        igo = 0  # idx16 column offset
        iio = 0  # dstloc instance column offset
        cgo = 0  # gbuf chunk offset (layer-wide, for naming only)
        for b in range(c.NB):
            tiles = list(range(b * c.T_B, min((b + 1) * c.T_B, c.NT)))
            gmeta = []
            cbase = 0
            for q in range(n_groups):
                # dense tile intervals
                ivals = []  # (t, lo, hi) in group-local slot coords
                s0 = 0
                for t in tiles:
                    r = int(self.real[t, q])
                    if r:
                        ivals.append((t, s0, s0 + r))
                    s0 += r
                tot_real = s0
                TOT = _r128(tot_real)
                nch = TOT // P
                wins = []
                for w0 in range(0, TOT, WIN):
                    nw = min(WIN, TOT - w0)
                    reg = min(nw, tot_real - w0)
                    if reg > 0:
                        wins.append((w0, nw, reg))
                insts = []  # (t, local_chunk, lo, hi)
                for ch in range(nch):
                    clo, chi = ch * P, ch * P + P
                    for (t, lo, hi) in ivals:
                        if lo < chi and hi > clo:
                            insts.append((t, ch, max(lo, clo), min(hi, chi)))
                gmeta.append(
                    dict(q=q, tot_real=tot_real, TOT=TOT, nch=nch,
                         cbase=cbase, wins=wins, ivals=ivals, insts=insts)
                )
                cbase += nch
            ninst = sum(len(g["insts"]) for g in gmeta)
            icols = sum(g["TOT"] // 16 for g in gmeta)
            self.batches.append(
                dict(tiles=tiles, groups=gmeta, nch=cbase, ninst=ninst,
                     igo=igo, iio=iio, cgo=cgo, icols=icols)
            )
            igo += icols
            iio += ninst
            cgo += cbase
        self.IGOT = igo
        self.NINST = iio
        self.MAXNCH = max(b["nch"] for b in self.batches)
        self.MAXNINST = max(b["ninst"] for b in self.batches)

    def core_arrays(self, ci):
        c = self.cfg
        idx16 = np.zeros((P, self.IGOT), dtype=np.int16)
        dstloc = np.full((P, self.NINST), -1.0, dtype=np.float16)
        g = self.groups[ci]
        for b in self.batches:
            icol = b["igo"]
            inst_col = b["iio"]
            for gm in b["groups"]:
                TOT = gm["TOT"]
                if TOT == 0:
                    continue
                q = gm["q"]
                sl_full = np.full(TOT, -1, dtype=np.int16)
                dl_full = np.full(TOT, -1, dtype=np.int16)
                for (t, lo, hi) in gm["ivals"]:
                    if (t, q) in g:
                        sl, dl = g[(t, q)]
                        n = len(sl)
                        sl_full[lo : lo + n] = sl
                        dl_full[lo : lo + n] = dl
                        if lo + n < hi:  # this core's pad within shared real
                            sl_full[lo + n : hi] = 0
                    else:
                        sl_full[lo:hi] = 0
                blk = sl_full.reshape(TOT // 16, 16).T
                idx16[:, icol : icol + TOT // 16] = np.tile(blk, (8, 1))
                icol += TOT // 16
                for (t, ch, lo, hi) in gm["insts"]:
                    col = np.full(P, -1.0, dtype=np.float16)
                    seg = dl_full[ch * P : ch * P + P]
                    mask = np.zeros(P, dtype=bool)
                    mask[lo - ch * P : hi - ch * P] = True
                    col[mask] = seg[mask].astype(np.float16)
                    dstloc[:, inst_col] = col
                    inst_col += 1
        return idx16, dstloc


class Plan:
    def __init__(self, cfg, src, dst):
        c = cfg
        self.cfg = c
        core = dst // c.SHARD
        e1, e2 = [], []
        for ci in range(c.NCORE):
            m = core == ci
            s = src[m]
            d = dst[m] - ci * c.SHARD
            t = d // P
            dl = d % P
            # layer 1: x16 table grouped by quarters of node id
            e1.append((t, s // c.QS, s % c.QS, dl))
            # layer 2: z_full is [slice][core][row]; slice = shard-row/SROW
            sc = s // c.SHARD
            sr = s % c.SHARD
            sl = sr // c.SROW
            lidx = sc * c.SROW + (sr - sl * c.SROW)
            e2.append((t, sl, lidx, dl))
        self.L1 = LayerPlan(cfg, c.NQ, e1)
        self.L2 = LayerPlan(cfg, c.NS, e2)

    def core_arrays(self, ci):
        i1, d1 = self.L1.core_arrays(ci)
        i2, d2 = self.L2.core_arrays(ci)
        return np.concatenate(
            [i1, d1.view(np.int16), i2, d2.view(np.int16)], axis=1
        )


def _build(plan):
    import concourse.tile as tile
    from concourse import bacc, mybir

    c = plan.cfg
    f16, f32, i16 = mybir.dt.float16, mybir.dt.float32, mybir.dt.int16
    CH, H, QS, SHARD = c.CH, c.H, c.QS, c.SHARD
    L1, L2 = plan.L1, plan.L2
    PKW = L1.IGOT + L1.NINST + L2.IGOT + L2.NINST

    nc = bacc.Bacc(
        "TRN2", target_bir_lowering=False, debug=False, num_devices=c.NCORE,
        num_swdge_queues=4, dynamic_dma_scratch_size=32768,
    )
    x16 = nc.dram_tensor("x16", [c.N, CH], f16, kind="ExternalInput")
    xT = nc.dram_tensor("xT", [CH, SHARD], f32, kind="ExternalInput")
    pk = nc.dram_tensor("pk", [P, PKW], i16, kind="ExternalInput")
    iota = nc.dram_tensor("iota", [P, P], f16, kind="ExternalInput")
    ident = nc.dram_tensor("ident", [H, H], f16, kind="ExternalInput")
    w1 = nc.dram_tensor("w1", [CH, H], f16, kind="ExternalInput")
    w2 = nc.dram_tensor("w2", [H, H], f16, kind="ExternalInput")
    w3 = nc.dram_tensor("w3", [H, 16], f16, kind="ExternalInput")
    w4 = nc.dram_tensor("w4", [16, 1], f16, kind="ExternalInput")
    b1 = nc.dram_tensor("b1", [H, 1], f32, kind="ExternalInput")
    b2 = nc.dram_tensor("b2", [H, 1], f32, kind="ExternalInput")
    b3 = nc.dram_tensor("b3", [16, 1], f32, kind="ExternalInput")
    b4v = nc.dram_tensor("b4v", [1, 1], f32, kind="ExternalInput")
    out = nc.dram_tensor("out", [1, SHARD], f32, kind="ExternalOutput")

    with tile.TileContext(nc) as tc:
        with (
            tc.tile_pool(name="const", bufs=1) as cp,
            tc.tile_pool(name="persist", bufs=1) as pp,
            tc.tile_pool(name="dram", bufs=1, space="DRAM") as dp,
        ):
            def load_const(name, t, shape, dt):
                sb = cp.tile(shape, dt, name=name + "_sb")
                nc.sync.dma_start(out=sb[:], in_=t[:, :])
                return sb

            iota_sb = load_const("iota", iota, [P, P], f16)
            id_sb = load_const("id", ident, [H, H], f16)
            w1_sb = load_const("w1", w1, [CH, H], f16)
            w2_sb = load_const("w2", w2, [H, H], f16)
            w3_sb = load_const("w3", w3, [H, 16], f16)
            w4_sb = load_const("w4", w4, [16, 1], f16)
            b1_sb = load_const("b1", b1, [H, 1], f32)
            b2_sb = load_const("b2", b2, [H, 1], f32)
            b3_sb = load_const("b3", b3, [16, 1], f32)
            b4_sb = load_const("b4", b4v, [1, 1], f32)

            zT_sh = pp.tile([H, SHARD], f16)
            outT = pp.tile([1, SHARD], f32)
            # per-slice DRAM tiles so phase-2 gathers of slice s depend only
            # on AllGather s (not the last one)
            SR8_ = c.SROW * c.NCORE
            z_shard = [
                dp.tile([c.SROW, P], f16, name=f"zs{s}") for s in range(c.NS)
            ]
            z_full = [
                dp.tile([SR8_, P], f16, name=f"zfull{s}") for s in range(c.NS)
            ]  # per slice: [core][row][ch]

            relu = mybir.ActivationFunctionType.Relu
            qrr = [0]  # round-robin SWDGE queue
            state = {}

            def tile_cols(t):
                return min(P, SHARD - t * P)

            def gather_pk(lp, b, pk_base):
                sm = state["sm"]
                icols = b["icols"]
                ninst = b["ninst"]
                pk_sb = sm.tile([P, icols + ninst], i16, tag="pk")
                nc.sync.dma_start(
                    out=pk_sb[:, :icols],
                    in_=pk[:, pk_base + b["igo"] : pk_base + b["igo"] + icols],
                )
                nc.sync.dma_start(
                    out=pk_sb[:, icols:],
                    in_=pk[
                        :,
                        pk_base + lp.IGOT + b["iio"] :
                        pk_base + lp.IGOT + b["iio"] + ninst,
                    ],
                )
                return pk_sb

            def gather_groups(b, pk_sb, gbuf, table_of, elem, qsel=None):
                ic = 0
                for gm in b["groups"]:
                    if gm["TOT"] == 0:
                        continue
                    if qsel is None or gm["q"] in qsel:
                        tbl = table_of(gm["q"])
                        for (w0, nw, reg) in gm["wins"]:
                            c0 = gm["cbase"] + w0 // P
                            nc.gpsimd.dma_gather(
                                gbuf[:, c0 : c0 + nw // P, :],
                                tbl,
                                pk_sb[:, ic + w0 // 16 : ic + (w0 + nw) // 16],
                                nw, reg, elem,
                                queue_num=qrr[0] % 4,
                                single_packet=False,
                            )
                            qrr[0] += 1
                    ic += gm["TOT"] // 16

            def gather_batch(lp, b, pk_base, gbuf, table_of, elem):
                pk_sb = gather_pk(lp, b, pk_base)
                gather_groups(b, pk_sb, gbuf, table_of, elem)
                return pk_sb[:, b["icols"] :].bitcast(f16)

            def onehot_and_agg(b, dl_sb, gbuf, ohp, psa, m_dim, tag, lw):
                ninst = b["ninst"]
                oh = ohp.tile([P, ninst, P], f16, tag="oh", name="oh")
                nc.vector.tensor_tensor(
                    out=oh[:],
                    in0=dl_sb.unsqueeze(2).to_broadcast([P, ninst, P]),
                    in1=iota_sb[:].unsqueeze(1).to_broadcast([P, ninst, P]),
                    op=mybir.AluOpType.is_equal,
                )
                mm = []  # (t, global chunk col, inst col)
                ii = 0
                for gm in b["groups"]:
                    for (t, ch, lo, hi) in gm["insts"]:
                        mm.append((t, gm["cbase"] + ch, ii))
                        ii += 1
                first, last, aggs = {}, {}, {}
                for i, (t, chc, ic_) in enumerate(mm):
                    first.setdefault(t, i)
                    last[t] = i
                for t in b["tiles"]:
                    aggs[t] = psa.tile(
                        [m_dim, P], f32, tag=tag, name=f"{tag}_{t}"
                    )
                    if t not in first:
                        nc.vector.memset(aggs[t][:], 0)
                for i, (t, chc, ic_) in enumerate(mm):
                    nc.tensor.matmul(
                        out=aggs[t][:],
                        lhsT=gbuf[:, chc : chc + 1, :lw],
                        rhs=oh[:, ic_ : ic_ + 1, :],
                        start=(i == first[t]),
                        stop=(i == last[t]),
                    )
                return aggs

            # shared pools across both phases: no close/open barrier, so
            # phase-2 gathers of already-allgathered z slices overlap the
            # phase-1 tail (last batches' MLP + final AG slice)
            GMAX = max(L1.MAXNCH, L2.MAXNCH)
            with (
                tc.tile_pool(name="g", bufs=3) as gp,
                tc.tile_pool(name="oh", bufs=1) as ohp,
                tc.tile_pool(name="sm", bufs=3) as sm,
                tc.tile_pool(name="ps", bufs=c.T_B, space="PSUM") as psa,
                tc.tile_pool(name="pst", bufs=1, space="PSUM") as pst,
            ):
                state["sm"] = sm
                ag_row = [0]

                def issue_ag(r1):
                    # one collective per completed slice: the AG output
                    # (concat over cores) is contiguous only within a slice
                    r1 = min(r1, SHARD)
                    while ag_row[0] + c.SROW <= r1:
                        s = ag_row[0] // c.SROW
                        nc.gpsimd.collective_compute(
                            "AllGather",
                            mybir.AluOpType.bypass,
                            replica_groups=[list(range(c.NCORE))],
                            ins=[z_shard[s][:, :].opt()],
                            outs=[z_full[s][:, :].opt()],
                        )
                        ag_row[0] += c.SROW

                for bi, b in enumerate(L1.batches):
                    if b["nch"] == 0:
                        continue
                    g1 = gp.tile([P, GMAX, CH], f16, tag="g", name="g1")
                    if bi < 3:  # must cover every rotating buffer of the g pool
                        nc.vector.memset(g1[:], 0)
                    dl_sb = gather_batch(
                        L1, b, 0, g1,
                        lambda q: x16[q * QS : (q + 1) * QS, :], CH,
                    )
                    aggs = onehot_and_agg(
                        b, dl_sb, g1, ohp, psa, CH, "agg", CH
                    )
                    for t in b["tiles"]:
                        tw = tile_cols(t)
                        xT_sb = sm.tile([CH, P], f32, tag="xt", name="xt")
                        nc.scalar.dma_start(
                            out=xT_sb[:, :tw], in_=xT[:, t * P : t * P + tw]
                        )
                        sT = sm.tile([CH, P], f16, tag="st", name="st")
                        nc.vector.tensor_add(
                            out=sT[:, :tw], in0=aggs[t][:, :tw],
                            in1=xT_sb[:, :tw],
                        )
                        h1p = pst.tile([H, P], f32, tag="h1", name="h1p")
                        nc.tensor.matmul(
                            out=h1p[:, :tw], lhsT=w1_sb[:], rhs=sT[:, :tw],
                            start=True, stop=True,
                        )
                        h1f = sm.tile([H, P], f16, tag="h1f", name="h1f")
                        nc.scalar.activation(
                            out=h1f[:, :tw], in_=h1p[:, :tw], func=relu,
                            bias=b1_sb[:],
                        )
                        zp = pst.tile([H, P], f32, tag="zp", name="zp")
                        nc.tensor.matmul(
                            out=zp[:, :tw], lhsT=w2_sb[:], rhs=h1f[:, :tw],
                            start=True, stop=True,
                        )
                        # copies ride the idle Scalar engine so Vector (is_eq
                        # + adds) can't stall the gather pipeline
                        nc.scalar.activation(
                            out=zT_sh[:, t * P : t * P + tw], in_=zp[:, :tw],
                            func=mybir.ActivationFunctionType.Copy,
                        )
                        zf = sm.tile([H, P], f16, tag="zf", name="zf")
                        nc.scalar.activation(
                            out=zf[:, :tw], in_=zp[:, :tw],
                            func=mybir.ActivationFunctionType.Copy,
                        )
                        ztr = pst.tile([P, H], f16, tag="ztr", name="ztr")
                        nc.tensor.transpose(
                            out=ztr[:tw, :], in_=zf[:, :tw], identity=id_sb[:]
                        )
                        # upper 64 ch of z rows are never read (L2 agg matmul
                        # uses lhsT[:, :H]) — leave them as garbage
                        zr = sm.tile([P, P], f16, tag="zr", name="zr")
                        nc.scalar.activation(
                            out=zr[:tw, :H], in_=ztr[:tw, :],
                            func=mybir.ActivationFunctionType.Copy,
                        )
                        rr, r1 = t * P, t * P + tw
                        while rr < r1:
                            s = rr // c.SROW
                            re = min(r1, (s + 1) * c.SROW)
                            nc.scalar.dma_start(
                                out=z_shard[s][
                                    rr - s * c.SROW : re - s * c.SROW, :
                                ],
                                in_=zr[rr - t * P : re - t * P, :],
                            )
                            rr = re
                    done_rows = (b["tiles"][-1] + 1) * P
                    issue_ag((done_rows // c.SROW) * c.SROW)
                issue_ag(SHARD)

                # ---------------- phase 2 (same pools) ----------------
                pk2 = L1.IGOT + L1.NINST
                zt_of = lambda s: z_full[s][:, :]

                def l2_tail(b, aggs):
                    for t in b["tiles"]:
                        tw = tile_cols(t)
                        zc = sm.tile([H, P], f32, tag="zc", name="zc")
                        nc.vector.tensor_copy(
                            out=zc[:, :tw], in_=zT_sh[:, t * P : t * P + tw]
                        )
                        t2 = sm.tile([H, P], f32, tag="t2", name="t2")
                        nc.vector.tensor_add(
                            out=t2[:, :tw], in0=aggs[t][:, :tw],
                            in1=zc[:, :tw],
                        )
                        h2f = sm.tile([H, P], f16, tag="h2f", name="h2f")
                        nc.scalar.activation(
                            out=h2f[:, :tw], in_=t2[:, :tw], func=relu,
                            bias=b2_sb[:],
                        )
                        h3p = pst.tile([16, P], f32, tag="h1", name="h3p")
                        nc.tensor.matmul(
                            out=h3p[:, :tw], lhsT=w3_sb[:], rhs=h2f[:, :tw],
                            start=True, stop=True,
                        )
                        h3f = sm.tile([16, P], f16, tag="h3f", name="h3f")
                        nc.scalar.activation(
                            out=h3f[:, :tw], in_=h3p[:, :tw], func=relu,
                            bias=b3_sb[:],
                        )
                        op_ = pst.tile([1, P], f32, tag="zp", name="op_")
                        nc.tensor.matmul(
                            out=op_[:, :tw], lhsT=w4_sb[:], rhs=h3f[:, :tw],
                            start=True, stop=True,
                        )
                        nc.vector.scalar_tensor_tensor(
                            out=outT[:, t * P : t * P + tw],
                            in0=op_[:, :tw],
                            scalar=1.0,
                            in1=b4_sb[:].to_broadcast([1, tw]),
                            op0=mybir.AluOpType.mult,
                            op1=mybir.AluOpType.add,
                        )

                l2bs = [b for b in L2.batches if b["nch"]]
                # first two batches: defer slice-3 windows until after both
                # batches' slice-0..2 gathers, hiding the final AllGather
                # latency behind useful gather work
                head = []
                for b in l2bs[:2]:
                    g2 = gp.tile([P, GMAX, P], f16, tag="g", name="g2")
                    pk_sb = gather_pk(L2, b, pk2)
                    gather_groups(b, pk_sb, g2, zt_of, P, qsel=(0, 1, 2))
                    head.append((b, g2, pk_sb))
                for (b, g2, pk_sb) in head:
                    gather_groups(b, pk_sb, g2, zt_of, P, qsel=(3,))
                for (b, g2, pk_sb) in head:
                    dl_sb = pk_sb[:, b["icols"] :].bitcast(f16)
                    aggs = onehot_and_agg(b, dl_sb, g2, ohp, psa, H, "agg", H)
                    l2_tail(b, aggs)
                for b in l2bs[2:]:
                    g2 = gp.tile([P, GMAX, P], f16, tag="g", name="g2")
                    dl_sb = gather_batch(L2, b, pk2, g2, zt_of, P)
                    aggs = onehot_and_agg(b, dl_sb, g2, ohp, psa, H, "agg", H)
                    l2_tail(b, aggs)
            nc.sync.dma_start(out=out[:, :], in_=outT[:])
    nc.compile()
    return nc


def _in_maps(cfg, plan, x, W1, b1, W2, b2, W3, b3, W4, b4):
    c = cfg
    common = dict(
        x16=x.astype(np.float16),
        iota=np.broadcast_to(np.arange(P, dtype=np.float16), (P, P)).copy(),
        ident=np.eye(c.H, dtype=np.float16),
        w1=W1.astype(np.float16),
        w2=W2.astype(np.float16),
        w3=W3.astype(np.float16),
        w4=W4.astype(np.float16),
        b1=b1.reshape(-1, 1).astype(np.float32),
        b2=b2.reshape(-1, 1).astype(np.float32),
        b3=b3.reshape(-1, 1).astype(np.float32),
        b4v=b4.reshape(1, 1).astype(np.float32),
    )
    in_maps = []
    for ci in range(c.NCORE):
        pk_a = plan.core_arrays(ci)
        xT_a = np.ascontiguousarray(
            x[ci * c.SHARD : (ci + 1) * c.SHARD].T.astype(np.float32)
        )
        in_maps.append(dict(common, pk=pk_a, xT=xT_a))
    return in_maps


def _run(cfg, plan, nc, x, W1, b1, W2, b2, W3, b3, W4, b4, **kw):
    from concourse.bass_utils import run_bass_kernel_spmd

    c = cfg
    in_maps = _in_maps(cfg, plan, x, W1, b1, W2, b2, W3, b3, W4, b4)
    res = run_bass_kernel_spmd(nc, in_maps, core_ids=list(range(c.NCORE)), **kw)
    outs = [res.results[ci]["out"].reshape(-1) for ci in range(c.NCORE)]
    return np.concatenate(outs).reshape(-1, 1).astype(np.float32), res


def kernel(x, edge_index, W1, b1, W2, b2, W3, b3, W4, b4):
    cfg = FULL
    x = np.asarray(x, dtype=np.float32)
    src = np.asarray(edge_index[0], dtype=np.int64)
    dst = np.asarray(edge_index[1], dtype=np.int64)
    plan = Plan(cfg, src, dst)
    nc = _build(plan)
    out, _ = _run(
        cfg, plan, nc, x,
        np.asarray(W1), np.asarray(b1), np.asarray(W2), np.asarray(b2),
        np.asarray(W3), np.asarray(b3), np.asarray(W4), np.asarray(b4),
    )
    return out
